# revision 6
# baseline (speedup 1.0000x reference)
"""OHEM loss (region + affinity) on Trainium2 — 8 NeuronCores, SPMD data-parallel.

Math: for each pair (gt, pred) with shared conf_map,
    loss = (gt - pred)^2 * conf_map
    pos  = gt > 0.1 ; pos_num = sum(pos)
    neg_num = min(n - pos_num, 3 * pos_num)
    result  = (topk(neg_loss, neg_num).sum() + (loss*pos).sum()) / (neg_num + pos_num)
When neg_num == n - pos_num (the min picks the negative count, true whenever
pos fraction >= 0.25), the top-k covers every negative element, so
result == loss.sum() / n exactly. The device computes per-shard
sum(loss) and sum(sign(gt - 0.1)) (for the branch decision); the host
combines in float64 and falls back to an exact numpy evaluation in the
(never-taken-for-this-distribution) other branch.
"""

import os
import sys

import numpy as np

for _p in ("/opt/trn_rl_repo", os.path.expanduser("~/.axon_site/_ro/trn_rl_repo")):
    if os.path.isdir(_p) and _p not in sys.path:
        sys.path.insert(0, _p)

import concourse.tile as tile
from concourse import bacc, mybir
from concourse.bass_utils import run_bass_kernel_spmd

B, CH, H, W = 16, 1, 768, 768
NCORES = 8
N_FULL = B * CH * H * W            # 9_437_184
N_CORE = N_FULL // NCORES          # 1_179_648
P = 128
T = 4                              # tiles per tensor per core
F = N_CORE // (P * T)              # 2304 free-dim columns per tile
NEG_RATIO = 3.0
POS_MIN = 0.1
NAMES = ("gt_region", "pred_region", "gt_affinity", "pred_affinity", "conf_map")
F32 = mybir.dt.float32
NACC = 2 * T                       # acc columns: [l_r: t] [l_a: T+t]

_NC_CACHE = None
LAST_RESULTS = None                # exposed for test harness profiling


def _emit(tc, ins, out):
    nc = tc.nc

    with (
        tc.tile_pool(name="io", bufs=2) as io_pool,
        tc.tile_pool(name="scr", bufs=2) as scr_pool,
        tc.tile_pool(name="accp", bufs=1) as acc_pool,
    ):
        acc = acc_pool.tile([P, NACC], F32)
        pairs = (("gt_region", "pred_region", 0), ("gt_affinity", "pred_affinity", 1))
        for t in range(T):
            tl = {}
            for nm in NAMES:
                buf = io_pool.tile([P, F], F32, tag=nm)
                nc.gpsimd.dma_start(buf[:], ins[nm][t, :, :])
                tl[nm] = buf
            conf = tl["conf_map"]
            for gt_nm, pr_nm, pi in pairs:
                gt, pred = tl[gt_nm], tl[pr_nm]
                d = scr_pool.tile([P, F], F32, tag="d")
                nc.vector.tensor_sub(d[:], gt[:], pred[:])
                d2 = scr_pool.tile([P, F], F32, tag="d2")
                nc.scalar.square(d2[:], d[:])
                l = scr_pool.tile([P, F], F32, tag="l")
                nc.vector.tensor_mul(l[:], d2[:], conf[:])
                nc.vector.reduce_sum(
                    acc[:, pi * T + t : pi * T + t + 1], l[:],
                    axis=mybir.AxisListType.X,
                )
        nc.gpsimd.dma_start(out[:], acc[:])


def _build_nc():
    nc = bacc.Bacc("TRN2", target_bir_lowering=False, debug=False, num_devices=NCORES)
    ins = {
        nm: nc.dram_tensor(nm, [T, P, F], F32, kind="ExternalInput").ap()
        for nm in NAMES
    }
    out = nc.dram_tensor("out", [P, NACC], F32, kind="ExternalOutput").ap()
    with tile.TileContext(nc) as tc:
        _emit(tc, ins, out)
    nc.compile()
    return nc


def get_nc():
    global _NC_CACHE
    if _NC_CACHE is None:
        _NC_CACHE = _build_nc()
    return _NC_CACHE


def _reference_loss_numpy(gt, pred, conf):
    """Exact numpy replica of the reference _get_loss (fallback path)."""
    n = gt.size
    gt = gt.reshape(-1).astype(np.float32)
    pred = pred.reshape(-1).astype(np.float32)
    conf = conf.reshape(-1).astype(np.float32)
    pos = (gt > POS_MIN).astype(np.float32)
    pos_num = np.float32(pos.sum(dtype=np.float32))
    neg_num = np.float32(min(np.float32(n) - pos_num, np.float32(NEG_RATIO) * pos_num))
    loss = (gt - pred) ** 2 * conf
    pos_loss_sum = np.float32((loss * pos).sum(dtype=np.float32))
    neg_loss = loss * (1.0 - pos)
    k = int(neg_num)
    sorted_neg = np.sort(neg_loss)[::-1]
    topk = np.float32(sorted_neg[:k].sum(dtype=np.float32))
    return float((topk + pos_loss_sum) / (neg_num + pos_num))


def kernel(**inputs):
    global LAST_RESULTS
    nc = get_nc()
    arrs = {
        nm: np.ascontiguousarray(np.asarray(inputs[nm], dtype=np.float32))
        for nm in NAMES
    }
    shards = {nm: a.reshape(NCORES, T, P, F) for nm, a in arrs.items()}
    in_maps = [{nm: shards[nm][i] for nm in NAMES} for i in range(NCORES)]
    res = run_bass_kernel_spmd(nc, in_maps, core_ids=list(range(NCORES)))
    LAST_RESULTS = res
    accs = np.stack([np.asarray(r["out"], dtype=np.float64) for r in res.results])
    col = accs.sum(axis=(0, 1))  # (2T,)
    n = float(N_FULL)
    total = 0.0
    specs = (
        (col[0:T].sum(), "gt_region", "pred_region"),
        (col[T : 2 * T].sum(), "gt_affinity", "pred_affinity"),
    )
    for l_sum, gt_nm, pr_nm in specs:
        # Branch decision only (O(n) boolean count, host): which arm the
        # reference's min() takes. The heavy loss reduction ran on device.
        pos_num = float(np.count_nonzero(arrs[gt_nm] > POS_MIN))
        neg_avail = n - pos_num
        if neg_avail <= NEG_RATIO * pos_num:
            # min() picks the full negative count -> top-k sums every negative
            total += l_sum / n
        else:
            total += _reference_loss_numpy(arrs[gt_nm], arrs[pr_nm], arrs["conf_map"])
    return np.float32(total)


# revision 10
# speedup vs baseline: 1.2309x; 1.2309x over previous
"""OHEM loss (region + affinity) on Trainium2 — 8 NeuronCores, SPMD data-parallel.

Math: for each pair (gt, pred) with shared conf_map,
    loss = (gt - pred)^2 * conf_map
    pos  = gt > 0.1 ; pos_num = sum(pos)
    neg_num = min(n - pos_num, 3 * pos_num)
    result  = (topk(neg_loss, neg_num).sum() + (loss*pos).sum()) / (neg_num + pos_num)
When neg_num == n - pos_num (the min picks the negative count, true whenever
pos fraction >= 0.25), the top-k covers every negative element, so
result == loss.sum() / n exactly. The device computes the per-shard
sum(loss) partials; the host combines them in float64, decides the min()
branch with a cheap boolean count, and falls back to an exact numpy
evaluation in the (never-taken-for-this-distribution) other branch.
"""

import os
import sys

import numpy as np

for _p in ("/opt/trn_rl_repo", os.path.expanduser("~/.axon_site/_ro/trn_rl_repo")):
    if os.path.isdir(_p) and _p not in sys.path:
        sys.path.insert(0, _p)

import concourse.tile as tile
from concourse import bacc, mybir
from concourse.bass_utils import run_bass_kernel_spmd

B, CH, H, W = 16, 1, 768, 768
NCORES = 8
N_FULL = B * CH * H * W            # 9_437_184
N_CORE = N_FULL // NCORES          # 1_179_648
P = 128
T = 4                              # tiles per tensor per core
F = N_CORE // (P * T)              # 2304 free-dim columns per tile
NEG_RATIO = 3.0
POS_MIN = 0.1
NAMES = ("gt_region", "pred_region", "gt_affinity", "pred_affinity", "conf_map")
F32 = mybir.dt.float32
NACC = 2 * T                       # acc columns: [l_r: t] [l_a: T+t]

_NC_CACHE = None
LAST_RESULTS = None                # exposed for test harness profiling


def _emit(tc, ins, out):
    nc = tc.nc

    with (
        tc.tile_pool(name="io", bufs=2) as io_pool,
        tc.tile_pool(name="scr", bufs=2) as scr_pool,
        tc.tile_pool(name="accp", bufs=1) as acc_pool,
    ):
        acc = acc_pool.tile([P, NACC], F32)
        pairs = (("gt_region", "pred_region", 0), ("gt_affinity", "pred_affinity", 1))
        for t in range(T):
            tl = {}
            for nm in NAMES:
                buf = io_pool.tile([P, F], F32, tag=nm)
                nc.gpsimd.dma_start(buf[:], ins[nm][t, :, :])
                tl[nm] = buf
            conf = tl["conf_map"]
            for gt_nm, pr_nm, pi in pairs:
                gt, pred = tl[gt_nm], tl[pr_nm]
                d = scr_pool.tile([P, F], F32, tag="d")
                nc.vector.tensor_sub(d[:], gt[:], pred[:])
                d2 = scr_pool.tile([P, F], F32, tag="d2")
                nc.scalar.square(d2[:], d[:])
                # Fused (d2 * 1.0) * conf with accum_out = free-axis sum:
                # one DVE pass instead of mul + reduce.
                l = scr_pool.tile([P, F], F32, tag="l")
                nc.vector.scalar_tensor_tensor(
                    out=l[:], in0=d2[:], scalar=1.0, in1=conf[:],
                    op0=mybir.AluOpType.mult, op1=mybir.AluOpType.mult,
                    accum_out=acc[:, pi * T + t : pi * T + t + 1],
                )
        nc.gpsimd.dma_start(out[:], acc[:])


def _build_nc():
    nc = bacc.Bacc("TRN2", target_bir_lowering=False, debug=False, num_devices=NCORES)
    ins = {
        nm: nc.dram_tensor(nm, [T, P, F], F32, kind="ExternalInput").ap()
        for nm in NAMES
    }
    out = nc.dram_tensor("out", [P, NACC], F32, kind="ExternalOutput").ap()
    with tile.TileContext(nc) as tc:
        _emit(tc, ins, out)
    nc.compile()
    return nc


def get_nc():
    global _NC_CACHE
    if _NC_CACHE is None:
        _NC_CACHE = _build_nc()
    return _NC_CACHE


def _reference_loss_numpy(gt, pred, conf):
    """Exact numpy replica of the reference _get_loss (fallback path)."""
    n = gt.size
    gt = gt.reshape(-1).astype(np.float32)
    pred = pred.reshape(-1).astype(np.float32)
    conf = conf.reshape(-1).astype(np.float32)
    pos = (gt > POS_MIN).astype(np.float32)
    pos_num = np.float32(pos.sum(dtype=np.float32))
    neg_num = np.float32(min(np.float32(n) - pos_num, np.float32(NEG_RATIO) * pos_num))
    loss = (gt - pred) ** 2 * conf
    pos_loss_sum = np.float32((loss * pos).sum(dtype=np.float32))
    neg_loss = loss * (1.0 - pos)
    k = int(neg_num)
    sorted_neg = np.sort(neg_loss)[::-1]
    topk = np.float32(sorted_neg[:k].sum(dtype=np.float32))
    return float((topk + pos_loss_sum) / (neg_num + pos_num))


def kernel(**inputs):
    global LAST_RESULTS
    nc = get_nc()
    arrs = {
        nm: np.ascontiguousarray(np.asarray(inputs[nm], dtype=np.float32))
        for nm in NAMES
    }
    shards = {nm: a.reshape(NCORES, T, P, F) for nm, a in arrs.items()}
    in_maps = [{nm: shards[nm][i] for nm in NAMES} for i in range(NCORES)]
    res = run_bass_kernel_spmd(nc, in_maps, core_ids=list(range(NCORES)))
    LAST_RESULTS = res
    accs = np.stack([np.asarray(r["out"], dtype=np.float64) for r in res.results])
    col = accs.sum(axis=(0, 1))  # (2T,)
    n = float(N_FULL)
    total = 0.0
    specs = (
        (col[0:T].sum(), "gt_region", "pred_region"),
        (col[T : 2 * T].sum(), "gt_affinity", "pred_affinity"),
    )
    for l_sum, gt_nm, pr_nm in specs:
        # Branch decision only (O(n) boolean count, host): which arm the
        # reference's min() takes. The heavy loss reduction ran on device.
        pos_num = float(np.count_nonzero(arrs[gt_nm] > POS_MIN))
        neg_avail = n - pos_num
        if neg_avail <= NEG_RATIO * pos_num:
            # min() picks the full negative count -> top-k sums every negative
            total += l_sum / n
        else:
            total += _reference_loss_numpy(arrs[gt_nm], arrs[pr_nm], arrs["conf_map"])
    return np.float32(total)
